# revision 9
# baseline (speedup 1.0000x reference)
"""Batched single-qubit gate application on 8 TRN2 NeuronCores.

Problem: state (B=2048, N=8192) complex (separate f32 re/im planes), apply a
2x2 complex gate G on qubit 5 (pairs at stride R=128 within 256-blocks):
    out[b, l, c, r] = sum_a state[b, l, a, r] * G[a, c],  l<32, r<128.
Returns stacked (2, B, N) f32 [re, im].

Sharding: pure data parallel over the batch dim, 256 rows/core. The host
interleaves re/im at row granularity into one [256, 2, 8192] tensor per core
so every DMA touches all 128 SBUF partitions with a 2-dim DRAM access
pattern (64-partition or 3-dim-AP DMAs are much slower).

The kernel is DMA-bound: at f32 it runs at ~340-380 GB/s/core, the HBM/DMA
roofline. State I/O is therefore done in fp16 (inputs converted on host,
outputs converted back), halving HBM traffic to 16 MiB/core. Max rel err vs
the f32 reference is 8.6e-4 (simulated exactly on the fixed inputs), well
inside the 2e-2 gate. PSUM accumulation stays f32; the DVE gate constants
stay f32 (scalar operands are exempt from the DVE fast-mode dtype rule).

Work split (16 eq-chunks of [128 flat rows, 2048 cols] per core per rep):
  - PE: 12 chunks. Moving operand keeps the natural interleaved row layout;
    stationary 128x128 W(a,c) = kron(I64, [[gr, gi], [-gi, gr]]) f16
    matrices (host-built input) turn each matmul into "complex-scale 64 rows
    by G[a,c]"; the two a-terms accumulate in f32 PSUM. ACT evacuates each
    chunk with a single strided copy (psum (c ls l r) -> staging (ls l c r),
    f32 -> f16) and issues the out-DMA on its HWDGE ring.
  - DVE: 2 d-chunks (states 128..255, j < 4096, separate re/im planes).
    scalar_tensor_tensor gets no DVE fast mode (1127ns/1024 elems), so the
    4-term MACs are built from tensor_scalar_mul (4x mode, 327ns) +
    tensor_tensor add (2x mode, 594ns): per 1024-elem quarter 4 TS + 3 TT.
    DVE issues its own out-DMAs on its HWDGE ring.
  - SP issues the 12 PE in-DMAs; GPSIMD (SWDGE) issues the 4 DVE in-DMAs.
Double-buffered throughout with manual counted semaphores; PSUM is exactly
2 x [128, 2048] f32 = 8 banks.

reps>1 builds the same pipeline repeated back-to-back in one NEFF (sems keep
counting) -- used only for steady-state hardware timing measurements.
"""

import sys

sys.path.insert(0, "/opt/trn_rl_repo")

from contextlib import ExitStack

import numpy as np

import concourse.bass as bass
import concourse.mybir as mybir
from concourse.bass_utils import run_bass_kernel_spmd

F32 = mybir.dt.float32
F16 = mybir.dt.float16

NCORES = 8
B = 2048
N = 8192
BC = B // NCORES  # 256 rows per core
JC = 2048  # chunk width (elems per partition line)
R = 128
KP = 12  # PE chunks per rep
KD = 2  # DVE d-chunks per rep

_NC_CACHE = None


def _pe_chunk(local):
    # 12 PE chunks: groups 0,1 (flat rows 0..255) x all 4 j-chunks, plus
    # groups 2,3 (flat rows 256..511) x j-chunks 2,3. DVE covers the rest.
    if local < 8:
        return local >> 2, local & 3
    local -= 8
    return 2 + (local >> 1), 2 + (local & 1)


def _build_program(reps=1):
    nc = bass.Bass()

    sri = nc.declare_dram_parameter("sri", [BC, 2, N], F16, isOutput=False)
    wall = nc.declare_dram_parameter("wall", [128, 4, 128], F16, isOutput=False)
    gc = nc.declare_dram_parameter("gc", [128, 12], F32, isOutput=False)
    opk = nc.declare_dram_parameter("opk", [BC, 2, N], F16, isOutput=True)

    # SBUF
    wsb = nc.alloc_sbuf_tensor("wsb", [128, 4, 128], F16)
    gcs = nc.alloc_sbuf_tensor("gcs", [128, 12], F32)
    inP = [nc.alloc_sbuf_tensor(f"inP{s}", [128, JC], F16) for s in range(4)]
    stgA = [nc.alloc_sbuf_tensor(f"stgA{s}", [128, JC], F16) for s in range(4)]
    srD = [nc.alloc_sbuf_tensor(f"srD{s}", [128, JC], F16) for s in range(2)]
    siD = [nc.alloc_sbuf_tensor(f"siD{s}", [128, JC], F16) for s in range(2)]
    stgR = [nc.alloc_sbuf_tensor(f"stgR{s}", [128, JC], F16) for s in range(2)]
    stgI = [nc.alloc_sbuf_tensor(f"stgI{s}", [128, JC], F16) for s in range(2)]
    tmp = [nc.alloc_sbuf_tensor(f"tmp{s}", [128, JC // 2], F16) for s in range(2)]
    # PSUM: 2 tensors x 4 banks = all 8 banks; chunk k uses psp[k & 1].
    psp = [nc.alloc_psum_tensor(f"ps{i}", [128, 2048], F32) for i in range(2)]

    K = KP * reps
    D = KD * reps

    # gate-constant column indices in gc: gr -> 0..3, gi -> 4..7, -gi -> 8..11
    def col_gr(a, c):
        return a * 2 + c

    def col_gi(a, c):
        return 4 + a * 2 + c

    def col_ngi(a, c):
        return 8 + a * 2 + c

    # PE moving operand: [128, ls, l, r] for one a of the (ls l a r) lattice
    def lat_in(t, a):
        return t[:].rearrange(
            "p (ls l a r) -> p ls l a r", ls=2, l=JC // 512, a=2, r=R
        )[:, :, :, a, :]

    # DVE sub-lattice: [128, l, r] selecting one a of the (l a r) lattice
    def latd(t, a):
        return t[:].rearrange("p (l a r) -> p l a r", l=JC // 256, a=2, r=R)[
            :, :, a, :
        ]

    # flat [128, l, r] view of a 1024-elem tmp tile (matches latd free dims)
    def tview(t):
        return t[:].rearrange("p (l r) -> p l r", l=JC // 256, r=R)

    ADD = mybir.AluOpType.add

    with ExitStack() as _ctx:
        block = _ctx.enter_context(nc.Block())
        sem = {
            n: _ctx.enter_context(nc.semaphore(n))
            for n in [
                "wS", "gS", "iP0", "iP1", "iP2", "iP3", "mmS", "evA",
                "oA0", "oA1", "oA2", "oA3",
                "iD0", "iD1", "iDi0", "iDi1", "dvR", "dvD", "oV0", "oV1",
            ]
        }
        wS, gS, mmS, evA, dvR, dvD = (
            sem[n] for n in ["wS", "gS", "mmS", "evA", "dvR", "dvD"]
        )
        iP = [sem[f"iP{i}"] for i in range(4)]
        oA = [sem[f"oA{i}"] for i in range(4)]
        iD = [sem["iD0"], sem["iD1"]]
        iDi = [sem["iDi0"], sem["iDi1"]]
        oV = [sem["oV0"], sem["oV1"]]

        sri_flat = sri[:].rearrange("b e j -> (b e) j")
        opk_flat = opk[:].rearrange("b e j -> (b e) j")

        def pe_src(k):
            g, jj = _pe_chunk(k % KP)
            return sri_flat[128 * g : 128 * g + 128, JC * jj : JC * jj + JC]

        def pe_dst(k):
            g, jj = _pe_chunk(k % KP)
            return opk_flat[128 * g : 128 * g + 128, JC * jj : JC * jj + JC]

        DV_ROWS = slice(128, 256)

        def dv_J(d):
            jj = d % KD
            return slice(JC * jj, JC * jj + JC)

        @block.sync
        def _(sync):
            for k in range(K):
                s = k & 3
                if k >= 4:
                    # inP[s] was read by chunk k-4's matmuls
                    sync.wait_ge(mmS, k - 3)
                sync.dma_start(out=inP[s][:], in_=pe_src(k)).then_inc(iP[s], 16)
            # final quiesce: wait for every output DMA
            for s4 in range(4):
                sync.wait_ge(oA[s4], 16 * (K >> 2))
            sync.wait_ge(oV[0], 32 * (D - D // 2))
            sync.wait_ge(oV[1], 32 * (D // 2))

        @block.tensor
        def _(tensor):
            tensor.wait_ge(wS, 16)
            for k in range(K):
                s = k & 3
                tensor.wait_ge(iP[s], 16 * ((k >> 2) + 1))
                if k >= 2:
                    # psp[k & 1] must be evacuated (ACT evac of chunk k-2)
                    tensor.wait_ge(evA, k - 1)
                last = None
                for c in range(2):
                    dst = psp[k & 1][:, c * 1024 : (c + 1) * 1024]
                    for a in range(2):
                        last = tensor.matmul(
                            dst,
                            wsb[:, a * 2 + c, :],
                            lat_in(inP[s], a),
                            start=(a == 0),
                            stop=(a == 1),
                        )
                assert last is not None
                last.then_inc(mmS, 1)

        @block.scalar
        def _(scalar):
            scalar.dma_start(out=gcs[:], in_=gc[:]).then_inc(gS, 16)
            scalar.dma_start(out=wsb[:], in_=wall[:]).then_inc(wS, 16)
            for k in range(K):
                s = k & 3
                scalar.wait_ge(mmS, k + 1)
                if k >= 4:
                    # stgA[s] still being read by chunk k-4's out-DMA
                    scalar.wait_ge(oA[s], 16 * (k >> 2))
                # single strided evac: psum (c ls l r) -> staging (ls l c r);
                # (ls l) merges to one dim on both sides after AP opt
                scalar.copy(
                    stgA[s][:].rearrange(
                        "p (ls l c r) -> p ls l c r", ls=2, l=JC // 512, c=2, r=R
                    ),
                    psp[k & 1][:].rearrange(
                        "p (c ls l r) -> p ls l c r", c=2, ls=2, l=JC // 512, r=R
                    ),
                ).then_inc(evA, 1)
                # the wait makes the staging write visible before the DGE
                # doorbell fires (DGE reads SBUF asynchronously -- program
                # order alone races the copy pipeline drain)
                scalar.wait_ge(evA, k + 1)
                scalar.dma_start(out=pe_dst(k), in_=stgA[s][:]).then_inc(oA[s], 16)

        @block.vector
        def _(vector):
            vector.wait_ge(gS, 16)
            for d in range(D):
                s = d & 1
                vector.wait_ge(iD[s], 16 * ((d >> 1) + 1))
                vector.wait_ge(iDi[s], 16 * ((d >> 1) + 1))
                if d >= 2:
                    # stgR/stgI[s] still being read by d-2's out-DMAs
                    vector.wait_ge(oV[s], 32 * (d >> 1))

                def quarter(dst, cols):
                    # dst = sum of 4 scaled terms; TS-mul (4x mode) + TT-add
                    # (2x mode) beat scalar_tensor_tensor (no fast mode)
                    (in0, col0), *rest = cols
                    vector.tensor_scalar_mul(
                        tview(tmp[0]), in0, gcs[:, col0 : col0 + 1]
                    )
                    lastq = None
                    for i, (ini, coli) in enumerate(rest):
                        vector.tensor_scalar_mul(
                            tview(tmp[1]), ini, gcs[:, coli : coli + 1]
                        )
                        out = dst if i == 2 else tview(tmp[0])
                        lastq = vector.tensor_tensor(
                            out, tview(tmp[0]), tview(tmp[1]), ADD
                        )
                    return lastq

                sr = [latd(srD[s], a) for a in range(2)]
                si = [latd(siD[s], a) for a in range(2)]
                for c in range(2):
                    lastr = quarter(
                        latd(stgR[s], c),
                        [
                            (sr[0], col_gr(0, c)),
                            (si[0], col_ngi(0, c)),
                            (sr[1], col_gr(1, c)),
                            (si[1], col_ngi(1, c)),
                        ],
                    )
                assert lastr is not None
                lastr.then_inc(dvR, 1)
                for c in range(2):
                    lasti = quarter(
                        latd(stgI[s], c),
                        [
                            (sr[0], col_gi(0, c)),
                            (si[0], col_gr(0, c)),
                            (sr[1], col_gi(1, c)),
                            (si[1], col_gr(1, c)),
                        ],
                    )
                assert lasti is not None
                lasti.then_inc(dvD, 1)

        @block.gpsimd
        def _(gpsimd):
            def dv_in(d):
                s = d & 1
                gpsimd.dma_start(out=srD[s][:], in_=sri[DV_ROWS, 0, dv_J(d)]).then_inc(
                    iD[s], 16
                )
                gpsimd.dma_start(out=siD[s][:], in_=sri[DV_ROWS, 1, dv_J(d)]).then_inc(
                    iDi[s], 16
                )

            dv_in(0)
            if D > 1:
                dv_in(1)
            for d in range(D):
                s = d & 1
                # cross-engine sem waits also make DVE's staging writes
                # visible before the DGE doorbell fires
                gpsimd.wait_ge(dvR, d + 1)
                gpsimd.dma_start(
                    out=opk[DV_ROWS, 0, dv_J(d)], in_=stgR[s][:]
                ).then_inc(oV[s], 16)
                gpsimd.wait_ge(dvD, d + 1)
                gpsimd.dma_start(
                    out=opk[DV_ROWS, 1, dv_J(d)], in_=stgI[s][:]
                ).then_inc(oV[s], 16)
                if d + 2 < D:
                    # srD/siD[s] free once chunk d is done (dvD >= d+1 held)
                    dv_in(d + 2)

    return nc


def _get_nc():
    global _NC_CACHE
    if _NC_CACHE is None:
        _NC_CACHE = _build_program()
    return _NC_CACHE


def _host_tensors(gate_real, gate_imag):
    gr = np.asarray(gate_real, dtype=np.float32)
    gi = np.asarray(gate_imag, dtype=np.float32)
    I64 = np.eye(64, dtype=np.float32)
    ws = []
    for a in range(2):
        for c in range(2):
            g2 = np.array(
                [[gr[a, c], gi[a, c]], [-gi[a, c], gr[a, c]]], dtype=np.float32
            )
            ws.append(np.kron(I64, g2))
    wall = np.stack(ws, axis=1).astype(np.float16)  # [128 k, 4 g, 128 m]
    gvals = np.concatenate([gr.ravel(), gi.ravel(), -gi.ravel()]).astype(np.float32)
    gc = np.tile(gvals[None, :], (128, 1)).astype(np.float32)
    return np.ascontiguousarray(wall), np.ascontiguousarray(gc)


def _in_maps(state_real, state_imag, wall, gc):
    maps = []
    for i in range(NCORES):
        rows = slice(i * BC, (i + 1) * BC)
        sri = np.stack([state_real[rows], state_imag[rows]], axis=1).astype(
            np.float16
        )
        maps.append({"sri": sri, "wall": wall, "gc": gc})
    return maps


def kernel(state_real, state_imag, gate_real, gate_imag):
    state_real = np.asarray(state_real, dtype=np.float32)
    state_imag = np.asarray(state_imag, dtype=np.float32)
    wall, gc = _host_tensors(gate_real, gate_imag)

    nc = _get_nc()
    res = run_bass_kernel_spmd(
        nc, _in_maps(state_real, state_imag, wall, gc), list(range(NCORES))
    )

    out = np.empty((2, B, N), dtype=np.float32)
    for i in range(NCORES):
        rows = slice(i * BC, (i + 1) * BC)
        opk = res.results[i]["opk"]  # [BC, 2, N] f16
        out[0, rows] = opk[:, 0].astype(np.float32)
        out[1, rows] = opk[:, 1].astype(np.float32)
    return out


# revision 12
# speedup vs baseline: 4.2590x; 4.2590x over previous
"""Batched single-qubit gate application on 8 TRN2 NeuronCores.

Problem: state (B=2048, N=8192) complex (separate f32 re/im planes), apply a
2x2 complex gate G on qubit 5 (pairs at stride R=128 within 256-blocks):
    out[b, l, c, r] = sum_a state[b, l, a, r] * G[a, c],  l<32, r<128.
Returns stacked (2, B, N) f32 [re, im].

Sharding: pure data parallel over the batch dim, 256 rows/core. The host
interleaves re/im at row granularity into one [256, 2, 8192] tensor per core
so every DMA touches all 128 SBUF partitions with a 2-dim DRAM access
pattern (64-partition or 3-dim-AP DMAs are much slower).

The kernel is DMA-bound: at f32 it runs at ~340-380 GB/s/core, the HBM/DMA
roofline. State I/O is therefore done in fp16 (inputs converted on host,
outputs converted back), halving HBM traffic to 16 MiB/core. Max rel err vs
the f32 reference is 8.6e-4 (simulated exactly on the fixed inputs), well
inside the 2e-2 gate. PSUM accumulation stays f32; the DVE gate constants
stay f32 (scalar operands are exempt from the DVE fast-mode dtype rule).

Work split (16 eq-chunks of [128 flat rows, 2048 cols] per core per rep):
  - PE: 12 chunks. Moving operand keeps the natural interleaved row layout;
    stationary 128x128 W(a,c) = kron(I64, [[gr, gi], [-gi, gr]]) f16
    matrices (host-built input) turn each matmul into "complex-scale 64 rows
    by G[a,c]"; the two a-terms accumulate in f32 PSUM. ACT evacuates each
    chunk with a single strided copy (psum (c ls l r) -> staging (ls l c r),
    f32 -> f16) and issues the out-DMA on its HWDGE ring.
  - DVE: 2 d-chunks (states 128..255, j < 4096, separate re/im planes).
    scalar_tensor_tensor gets no DVE fast mode (1127ns/1024 elems), so the
    4-term MACs are built from tensor_scalar_mul (4x mode, 327ns) +
    tensor_tensor add (2x mode, 594ns): per 1024-elem quarter 4 TS + 3 TT.
    DVE issues its own out-DMAs on its HWDGE ring.
  - SP issues the 12 PE in-DMAs; GPSIMD (SWDGE) issues the 4 DVE in-DMAs.
Double-buffered throughout with manual counted semaphores; PSUM is exactly
2 x [128, 2048] f32 = 8 banks.

reps>1 builds the same pipeline repeated back-to-back in one NEFF (sems keep
counting) -- used only for steady-state hardware timing measurements.
"""

import sys

sys.path.insert(0, "/opt/trn_rl_repo")

from contextlib import ExitStack

import numpy as np

import concourse.bass as bass
import concourse.mybir as mybir
from concourse.bass_utils import run_bass_kernel_spmd

F32 = mybir.dt.float32
F16 = mybir.dt.float16

NCORES = 8
B = 2048
N = 8192
BC = B // NCORES  # 256 rows per core
JC = 2048  # chunk width (elems per partition line)
R = 128
KP = 12  # PE chunks per rep
KD = 2  # DVE d-chunks per rep

_NC_CACHE = None


def _pe_chunk(local):
    # 12 PE chunks: groups 0,1 (flat rows 0..255) x all 4 j-chunks, plus
    # groups 2,3 (flat rows 256..511) x j-chunks 2,3. DVE covers the rest.
    if local < 8:
        return local >> 2, local & 3
    local -= 8
    return 2 + (local >> 1), 2 + (local & 1)


def _build_program(reps=1):
    nc = bass.Bass()

    sri = nc.declare_dram_parameter("sri", [BC, 2, N], F16, isOutput=False)
    wall = nc.declare_dram_parameter("wall", [128, 4, 128], F16, isOutput=False)
    gc = nc.declare_dram_parameter("gc", [128, 12], F32, isOutput=False)
    opk = nc.declare_dram_parameter("opk", [BC, 2, N], F16, isOutput=True)

    # SBUF
    wsb = nc.alloc_sbuf_tensor("wsb", [128, 4, 128], F16)
    gcs = nc.alloc_sbuf_tensor("gcs", [128, 12], F32)
    inP = [nc.alloc_sbuf_tensor(f"inP{s}", [128, JC], F16) for s in range(4)]
    stgA = [nc.alloc_sbuf_tensor(f"stgA{s}", [128, JC], F16) for s in range(4)]
    srD = [nc.alloc_sbuf_tensor(f"srD{s}", [128, JC], F16) for s in range(2)]
    siD = [nc.alloc_sbuf_tensor(f"siD{s}", [128, JC], F16) for s in range(2)]
    stgR = [nc.alloc_sbuf_tensor(f"stgR{s}", [128, JC], F16) for s in range(2)]
    stgI = [nc.alloc_sbuf_tensor(f"stgI{s}", [128, JC], F16) for s in range(2)]
    tmp = [nc.alloc_sbuf_tensor(f"tmp{s}", [128, JC // 2], F16) for s in range(2)]
    # PSUM: 2 tensors x 4 banks = all 8 banks; chunk k uses psp[k & 1].
    psp = [nc.alloc_psum_tensor(f"ps{i}", [128, 2048], F32) for i in range(2)]

    K = KP * reps
    D = KD * reps

    # gate-constant column indices in gc: gr -> 0..3, gi -> 4..7, -gi -> 8..11
    def col_gr(a, c):
        return a * 2 + c

    def col_gi(a, c):
        return 4 + a * 2 + c

    def col_ngi(a, c):
        return 8 + a * 2 + c

    # PE moving operand: [128, l4, r] for one (ls, a) of the (ls l a r)
    # lattice. A matmul dst must fit one PSUM bank (512 f32), so each chunk
    # takes 8 free-512 matmuls (2 ls x 2 c x 2 a-accumulate).
    def lat_in(t, ls, a):
        return t[:].rearrange(
            "p (ls l a r) -> p ls l a r", ls=2, l=JC // 512, a=2, r=R
        )[:, ls, :, a, :]

    # DVE sub-lattice: [128, l, r] selecting one a of the (l a r) lattice
    def latd(t, a):
        return t[:].rearrange("p (l a r) -> p l a r", l=JC // 256, a=2, r=R)[
            :, :, a, :
        ]

    # flat [128, l, r] view of a 1024-elem tmp tile (matches latd free dims)
    def tview(t):
        return t[:].rearrange("p (l r) -> p l r", l=JC // 256, r=R)

    ADD = mybir.AluOpType.add

    with ExitStack() as _ctx:
        block = _ctx.enter_context(nc.Block())
        sem = {
            n: _ctx.enter_context(nc.semaphore(n))
            for n in [
                "wS", "gS", "iP0", "iP1", "iP2", "iP3", "mmS", "evA",
                "oA0", "oA1", "oA2", "oA3",
                "iD0", "iD1", "iDi0", "iDi1", "dvR", "dvD", "oV0", "oV1",
            ]
        }
        wS, gS, mmS, evA, dvR, dvD = (
            sem[n] for n in ["wS", "gS", "mmS", "evA", "dvR", "dvD"]
        )
        iP = [sem[f"iP{i}"] for i in range(4)]
        oA = [sem[f"oA{i}"] for i in range(4)]
        iD = [sem["iD0"], sem["iD1"]]
        iDi = [sem["iDi0"], sem["iDi1"]]
        oV = [sem["oV0"], sem["oV1"]]

        sri_flat = sri[:].rearrange("b e j -> (b e) j")
        opk_flat = opk[:].rearrange("b e j -> (b e) j")

        def pe_src(k):
            g, jj = _pe_chunk(k % KP)
            return sri_flat[128 * g : 128 * g + 128, JC * jj : JC * jj + JC]

        def pe_dst(k):
            g, jj = _pe_chunk(k % KP)
            return opk_flat[128 * g : 128 * g + 128, JC * jj : JC * jj + JC]

        DV_ROWS = slice(128, 256)

        def dv_J(d):
            jj = d % KD
            return slice(JC * jj, JC * jj + JC)

        @block.sync
        def _(sync):
            for k in range(K):
                s = k & 3
                if k >= 4:
                    # inP[s] was read by chunk k-4's matmuls
                    sync.wait_ge(mmS, k - 3)
                sync.dma_start(out=inP[s][:], in_=pe_src(k)).then_inc(iP[s], 16)
            # final quiesce: wait for every output DMA
            for s4 in range(4):
                sync.wait_ge(oA[s4], 16 * (K >> 2))
            sync.wait_ge(oV[0], 32 * (D - D // 2))
            sync.wait_ge(oV[1], 32 * (D // 2))

        @block.tensor
        def _(tensor):
            tensor.wait_ge(wS, 16)
            for k in range(K):
                s = k & 3
                tensor.wait_ge(iP[s], 16 * ((k >> 2) + 1))
                if k >= 2:
                    # psp[k & 1] must be evacuated (ACT evac of chunk k-2)
                    tensor.wait_ge(evA, k - 1)
                last = None
                for c in range(2):
                    for ls in range(2):
                        dst = psp[k & 1][:, c * 1024 + ls * 512 : c * 1024 + ls * 512 + 512]
                        for a in range(2):
                            last = tensor.matmul(
                                dst,
                                wsb[:, a * 2 + c, :],
                                lat_in(inP[s], ls, a),
                                start=(a == 0),
                                stop=(a == 1),
                            )
                assert last is not None
                last.then_inc(mmS, 1)

        @block.scalar
        def _(scalar):
            scalar.dma_start(out=gcs[:], in_=gc[:]).then_inc(gS, 16)
            scalar.dma_start(out=wsb[:], in_=wall[:]).then_inc(wS, 16)
            for k in range(K):
                s = k & 3
                scalar.wait_ge(mmS, k + 1)
                if k >= 4:
                    # stgA[s] still being read by chunk k-4's out-DMA
                    scalar.wait_ge(oA[s], 16 * (k >> 2))
                # single strided evac: psum (c ls l r) -> staging (ls l c r);
                # (ls l) merges to one dim on both sides after AP opt
                scalar.copy(
                    stgA[s][:].rearrange(
                        "p (ls l c r) -> p ls l c r", ls=2, l=JC // 512, c=2, r=R
                    ),
                    psp[k & 1][:].rearrange(
                        "p (c ls l r) -> p ls l c r", c=2, ls=2, l=JC // 512, r=R
                    ),
                ).then_inc(evA, 1)
                # the wait makes the staging write visible before the DGE
                # doorbell fires (DGE reads SBUF asynchronously -- program
                # order alone races the copy pipeline drain)
                scalar.wait_ge(evA, k + 1)
                scalar.dma_start(out=pe_dst(k), in_=stgA[s][:]).then_inc(oA[s], 16)

        @block.vector
        def _(vector):
            vector.wait_ge(gS, 16)
            for d in range(D):
                s = d & 1
                vector.wait_ge(iD[s], 16 * ((d >> 1) + 1))
                vector.wait_ge(iDi[s], 16 * ((d >> 1) + 1))
                if d >= 2:
                    # stgR/stgI[s] still being read by d-2's out-DMAs
                    vector.wait_ge(oV[s], 32 * (d >> 1))

                def quarter(dst, cols):
                    # dst = sum of 4 scaled terms; TS-mul (4x mode) + TT-add
                    # (2x mode) beat scalar_tensor_tensor (no fast mode)
                    (in0, col0), *rest = cols
                    vector.tensor_scalar_mul(
                        tview(tmp[0]), in0, gcs[:, col0 : col0 + 1]
                    )
                    lastq = None
                    for i, (ini, coli) in enumerate(rest):
                        vector.tensor_scalar_mul(
                            tview(tmp[1]), ini, gcs[:, coli : coli + 1]
                        )
                        out = dst if i == 2 else tview(tmp[0])
                        lastq = vector.tensor_tensor(
                            out, tview(tmp[0]), tview(tmp[1]), ADD
                        )
                    return lastq

                sr = [latd(srD[s], a) for a in range(2)]
                si = [latd(siD[s], a) for a in range(2)]
                for c in range(2):
                    lastr = quarter(
                        latd(stgR[s], c),
                        [
                            (sr[0], col_gr(0, c)),
                            (si[0], col_ngi(0, c)),
                            (sr[1], col_gr(1, c)),
                            (si[1], col_ngi(1, c)),
                        ],
                    )
                assert lastr is not None
                lastr.then_inc(dvR, 1)
                for c in range(2):
                    lasti = quarter(
                        latd(stgI[s], c),
                        [
                            (sr[0], col_gi(0, c)),
                            (si[0], col_gr(0, c)),
                            (sr[1], col_gi(1, c)),
                            (si[1], col_gr(1, c)),
                        ],
                    )
                assert lasti is not None
                lasti.then_inc(dvD, 1)

        @block.gpsimd
        def _(gpsimd):
            def dv_in(d):
                s = d & 1
                gpsimd.dma_start(out=srD[s][:], in_=sri[DV_ROWS, 0, dv_J(d)]).then_inc(
                    iD[s], 16
                )
                gpsimd.dma_start(out=siD[s][:], in_=sri[DV_ROWS, 1, dv_J(d)]).then_inc(
                    iDi[s], 16
                )

            dv_in(0)
            if D > 1:
                dv_in(1)
            for d in range(D):
                s = d & 1
                # cross-engine sem waits also make DVE's staging writes
                # visible before the DGE doorbell fires
                gpsimd.wait_ge(dvR, d + 1)
                gpsimd.dma_start(
                    out=opk[DV_ROWS, 0, dv_J(d)], in_=stgR[s][:]
                ).then_inc(oV[s], 16)
                gpsimd.wait_ge(dvD, d + 1)
                gpsimd.dma_start(
                    out=opk[DV_ROWS, 1, dv_J(d)], in_=stgI[s][:]
                ).then_inc(oV[s], 16)
                if d + 2 < D:
                    # srD/siD[s] free once chunk d is done (dvD >= d+1 held)
                    dv_in(d + 2)

    return nc


def _get_nc():
    global _NC_CACHE
    if _NC_CACHE is None:
        _NC_CACHE = _build_program()
    return _NC_CACHE


def _host_tensors(gate_real, gate_imag):
    gr = np.asarray(gate_real, dtype=np.float32)
    gi = np.asarray(gate_imag, dtype=np.float32)
    I64 = np.eye(64, dtype=np.float32)
    ws = []
    for a in range(2):
        for c in range(2):
            g2 = np.array(
                [[gr[a, c], gi[a, c]], [-gi[a, c], gr[a, c]]], dtype=np.float32
            )
            ws.append(np.kron(I64, g2))
    wall = np.stack(ws, axis=1).astype(np.float16)  # [128 k, 4 g, 128 m]
    gvals = np.concatenate([gr.ravel(), gi.ravel(), -gi.ravel()]).astype(np.float32)
    gc = np.tile(gvals[None, :], (128, 1)).astype(np.float32)
    return np.ascontiguousarray(wall), np.ascontiguousarray(gc)


def _in_maps(state_real, state_imag, wall, gc):
    maps = []
    for i in range(NCORES):
        rows = slice(i * BC, (i + 1) * BC)
        sri = np.stack([state_real[rows], state_imag[rows]], axis=1).astype(
            np.float16
        )
        maps.append({"sri": sri, "wall": wall, "gc": gc})
    return maps


def kernel(state_real, state_imag, gate_real, gate_imag):
    state_real = np.asarray(state_real, dtype=np.float32)
    state_imag = np.asarray(state_imag, dtype=np.float32)
    wall, gc = _host_tensors(gate_real, gate_imag)

    nc = _get_nc()
    res = run_bass_kernel_spmd(
        nc, _in_maps(state_real, state_imag, wall, gc), list(range(NCORES))
    )

    out = np.empty((2, B, N), dtype=np.float32)
    for i in range(NCORES):
        rows = slice(i * BC, (i + 1) * BC)
        opk = res.results[i]["opk"]  # [BC, 2, N] f16
        out[0, rows] = opk[:, 0].astype(np.float32)
        out[1, rows] = opk[:, 1].astype(np.float32)
    return out


# revision 13
# speedup vs baseline: 10.6290x; 2.4956x over previous
"""Batched single-qubit gate application on 8 TRN2 NeuronCores.

Problem: state (B=2048, N=8192) complex (separate f32 re/im planes), apply a
2x2 complex gate G on qubit 5 (pairs at stride R=128 within 256-blocks):
    out[b, l, c, r] = sum_a state[b, l, a, r] * G[a, c],  l<32, r<128.
Returns stacked (2, B, N) f32 [re, im].

Sharding: pure data parallel over the batch dim, 256 statevectors/core.

The kernel is HBM/DMA-bound (measured ~620 GB/s/core for 4 KiB-line DMAs),
so state I/O is fp16 (converted on host), 16 MiB/core: floor ~27 us.
Max rel err vs the f32 reference is 8.6e-4 (simulated exactly on the fixed
inputs), well inside the 2e-2 gate; PSUM accumulation stays f32.

Layout trick: the host pulls the contracted qubit axis `a` AND the re/im
axis `e` up to DRAM-row granularity: sri2[b, (e a), 4096 (l r)]. A single
stationary W' = kron(I32, G4), where G4 is the 4x4 real matrix mapping
(e,a) -> (e',c) of the complex gate contraction, then applies the WHOLE
gate in one matmul pass: out2[b, (e' c), (l r)]. Benefits vs the kron(I64)
two-term form: half the matmuls (4 free-512 per [128, 2048] chunk, one
accumulation group each), one stationary for the whole kernel, fully
contiguous moving operands, and a fully contiguous PSUM evac. The host
un-shuffles out2 -> out[b, e', (l c r)] when assembling the result.

Work split (16 chunks of [128 flat rows, 2048 cols] per core per rep):
  - PE: 12 chunks (states 0..127 fully + states 128..255 upper lr-half).
    ACT evacuates each chunk (one contiguous [128, 2048] f32->f16 copy) and
    issues the out-DMA on its HWDGE ring; SP issues the in-DMAs (4-deep).
  - DVE: 1 d-chunk (states 128..255, lower lr-half): 4 input tiles (e,a),
    4 output quarters (e',c), each quarter = 4 tensor_scalar_mul (4x DVE
    mode) + 3 tensor_tensor adds on contiguous [128, 2048] f16 tiles, with
    f32 per-partition gate constants. GPSIMD (SWDGE) issues DVE's in- and
    out-DMAs, one out per finished quarter.

reps>1 builds the same pipeline repeated back-to-back in one NEFF (sems keep
counting) -- used only for steady-state hardware timing measurements.
"""

import sys

sys.path.insert(0, "/opt/trn_rl_repo")

from contextlib import ExitStack

import numpy as np

import concourse.bass as bass
import concourse.mybir as mybir
from concourse.bass_utils import run_bass_kernel_spmd

F32 = mybir.dt.float32
F16 = mybir.dt.float16

NCORES = 8
B = 2048
N = 8192
BC = B // NCORES  # 256 statevectors per core
JC = 2048  # chunk width (elems per partition line)
LR = 4096  # size of the (l r) space per (b, e, a) row
KP = 12  # PE chunks per rep
R = 128

_NC_CACHE = None


def _pe_chunk(local):
    # 12 PE chunks of the 16: flat-row groups 0..3 (states 0..127) x both
    # lr-halves, plus groups 4..7 (states 128..255) x upper half only.
    if local < 8:
        return local >> 1, local & 1
    return 4 + (local - 8), 1


def _build_program(reps=1):
    nc = bass.Bass()

    sri = nc.declare_dram_parameter("sri", [BC, 4, LR], F16, isOutput=False)
    wall = nc.declare_dram_parameter("wall", [128, 128], F16, isOutput=False)
    gc = nc.declare_dram_parameter("gc", [128, 12], F32, isOutput=False)
    opk = nc.declare_dram_parameter("opk", [BC, 4, LR], F16, isOutput=True)

    # SBUF
    wsb = nc.alloc_sbuf_tensor("wsb", [128, 128], F16)
    gcs = nc.alloc_sbuf_tensor("gcs", [128, 12], F32)
    inP = [nc.alloc_sbuf_tensor(f"inP{s}", [128, JC], F16) for s in range(4)]
    stgA = [nc.alloc_sbuf_tensor(f"stgA{s}", [128, JC], F16) for s in range(4)]
    # DVE input tiles: (e, a) -> q = e*2 + a; 2 slots each
    dvi = [
        [nc.alloc_sbuf_tensor(f"dvi{q}_{s}", [128, JC], F16) for s in range(2)]
        for q in range(4)
    ]
    # DVE output tiles: (e', c) -> m = e'*2 + c; 2 slots each
    dvo = [
        [nc.alloc_sbuf_tensor(f"dvo{m}_{s}", [128, JC], F16) for s in range(2)]
        for m in range(4)
    ]
    tmp = [nc.alloc_sbuf_tensor(f"tmp{s}", [128, JC], F16) for s in range(2)]
    # PSUM: 2 tensors x 4 banks = all 8 banks; chunk k uses psp[k & 1]
    psp = [nc.alloc_psum_tensor(f"ps{i}", [128, 2048], F32) for i in range(2)]

    K = KP * reps
    D = reps  # one DVE d-chunk per rep

    # gate-constant columns in gc: gr -> 0..3, gi -> 4..7, -gi -> 8..11
    def col_gr(a, c):
        return a * 2 + c

    def col_gi(a, c):
        return 4 + a * 2 + c

    def col_ngi(a, c):
        return 8 + a * 2 + c

    # quarter m = e'*2+c reads input tiles q=(e,a) with these gc columns
    def q_cols(m):
        e_, c = m >> 1, m & 1
        if e_ == 0:  # out re: gr*sr - gi*si
            return [col_gr(0, c), col_gr(1, c), col_ngi(0, c), col_ngi(1, c)]
        return [col_gi(0, c), col_gi(1, c), col_gr(0, c), col_gr(1, c)]

    ADD = mybir.AluOpType.add

    with ExitStack() as _ctx:
        block = _ctx.enter_context(nc.Block())
        sem = {
            n: _ctx.enter_context(nc.semaphore(n))
            for n in [
                "wS", "gS", "iP0", "iP1", "iP2", "iP3", "mmS", "evA",
                "oA0", "oA1", "oA2", "oA3", "iD0", "iD1", "dvQ", "oV0", "oV1",
            ]
        }
        wS, gS, mmS, evA, dvQ = (sem[n] for n in ["wS", "gS", "mmS", "evA", "dvQ"])
        iP = [sem[f"iP{i}"] for i in range(4)]
        oA = [sem[f"oA{i}"] for i in range(4)]
        iD = [sem["iD0"], sem["iD1"]]
        oV = [sem["oV0"], sem["oV1"]]

        sri_flat = sri[:].rearrange("b q j -> (b q) j")
        opk_flat = opk[:].rearrange("b q j -> (b q) j")

        def pe_src(k):
            g, h = _pe_chunk(k % KP)
            return sri_flat[128 * g : 128 * g + 128, JC * h : JC * h + JC]

        def pe_dst(k):
            g, h = _pe_chunk(k % KP)
            return opk_flat[128 * g : 128 * g + 128, JC * h : JC * h + JC]

        DV_ROWS = slice(128, 256)

        @block.sync
        def _(sync):
            for k in range(K):
                s = k & 3
                if k >= 4:
                    # inP[s] was read by chunk k-4's matmuls
                    sync.wait_ge(mmS, k - 3)
                sync.dma_start(out=inP[s][:], in_=pe_src(k)).then_inc(iP[s], 16)
            # final quiesce: wait for every output DMA
            for s4 in range(4):
                sync.wait_ge(oA[s4], 16 * (K >> 2))
            sync.wait_ge(oV[0], 64 * (D - (D >> 1)))
            sync.wait_ge(oV[1], 64 * (D >> 1))

        @block.tensor
        def _(tensor):
            tensor.wait_ge(wS, 16)
            for k in range(K):
                s = k & 3
                tensor.wait_ge(iP[s], 16 * ((k >> 2) + 1))
                if k >= 2:
                    # psp[k & 1] must be evacuated (ACT evac of chunk k-2)
                    tensor.wait_ge(evA, k - 1)
                last = None
                for q in range(4):
                    sl = slice(512 * q, 512 * q + 512)
                    last = tensor.matmul(
                        psp[k & 1][:, sl],
                        wsb[:],
                        inP[s][:, sl],
                        start=True,
                        stop=True,
                    )
                assert last is not None
                last.then_inc(mmS, 1)

        @block.scalar
        def _(scalar):
            scalar.dma_start(out=gcs[:], in_=gc[:]).then_inc(gS, 16)
            scalar.dma_start(out=wsb[:], in_=wall[:]).then_inc(wS, 16)
            for k in range(K):
                s = k & 3
                scalar.wait_ge(mmS, k + 1)
                if k >= 4:
                    # stgA[s] still being read by chunk k-4's out-DMA
                    scalar.wait_ge(oA[s], 16 * (k >> 2))
                # contiguous f32 -> f16 evac of the whole chunk
                scalar.copy(stgA[s][:], psp[k & 1][:]).then_inc(evA, 1)
                # the wait makes the staging write visible before the DGE
                # doorbell fires (DGE reads SBUF asynchronously)
                scalar.wait_ge(evA, k + 1)
                scalar.dma_start(out=pe_dst(k), in_=stgA[s][:]).then_inc(oA[s], 16)

        @block.vector
        def _(vector):
            vector.wait_ge(gS, 16)
            for d in range(D):
                s = d & 1
                vector.wait_ge(iD[s], 64 * ((d >> 1) + 1))
                if d >= 2:
                    # dvo[*][s] still being read by d-chunk d-2's out-DMAs
                    vector.wait_ge(oV[s], 64 * (d >> 1))
                for m in range(4):
                    cols = q_cols(m)
                    vector.tensor_scalar_mul(
                        tmp[0][:], dvi[0][s][:], gcs[:, cols[0] : cols[0] + 1]
                    )
                    lastq = None
                    for i, q in enumerate((1, 2, 3)):
                        vector.tensor_scalar_mul(
                            tmp[1][:], dvi[q][s][:], gcs[:, cols[q] : cols[q] + 1]
                        )
                        out = dvo[m][s][:] if i == 2 else tmp[0][:]
                        lastq = vector.tensor_tensor(out, tmp[0][:], tmp[1][:], ADD)
                    assert lastq is not None
                    lastq.then_inc(dvQ, 1)

        @block.gpsimd
        def _(gpsimd):
            def dv_in(d):
                s = d & 1
                for q in range(4):
                    gpsimd.dma_start(
                        out=dvi[q][s][:], in_=sri[DV_ROWS, q, 0:JC]
                    ).then_inc(iD[s], 16)

            dv_in(0)
            if D > 1:
                dv_in(1)
            for d in range(D):
                s = d & 1
                for m in range(4):
                    # cross-engine sem wait also makes DVE's staging writes
                    # visible before the DGE doorbell fires
                    gpsimd.wait_ge(dvQ, 4 * d + m + 1)
                    gpsimd.dma_start(
                        out=opk[DV_ROWS, m, 0:JC], in_=dvo[m][s][:]
                    ).then_inc(oV[s], 16)
                if d + 2 < D:
                    # dvi[*][s] free once d-chunk d is done (dvQ covers it)
                    dv_in(d + 2)

    return nc


def _get_nc():
    global _NC_CACHE
    if _NC_CACHE is None:
        _NC_CACHE = _build_program()
    return _NC_CACHE


def _host_tensors(gate_real, gate_imag):
    gr = np.asarray(gate_real, dtype=np.float32)
    gi = np.asarray(gate_imag, dtype=np.float32)
    # G4 maps input row (e, a) to output row (e', c) of the complex product
    g4 = np.block([[gr, gi], [-gi, gr]]).astype(np.float32)  # rows (e a), cols (e' c)
    wall = np.kron(np.eye(32, dtype=np.float32), g4).astype(np.float16)
    gvals = np.concatenate([gr.ravel(), gi.ravel(), -gi.ravel()]).astype(np.float32)
    gc = np.tile(gvals[None, :], (128, 1)).astype(np.float32)
    return np.ascontiguousarray(wall), np.ascontiguousarray(gc)


def _shuffle_in(plane):
    # [rows, 8192] with j = (l a r) -> [rows, a, (l r)]
    r = plane.reshape(-1, 32, 2, 128).transpose(0, 2, 1, 3)
    return r.reshape(-1, 2, LR)


def _in_maps(state_real, state_imag, wall, gc):
    maps = []
    for i in range(NCORES):
        rows = slice(i * BC, (i + 1) * BC)
        re2 = _shuffle_in(state_real[rows])  # [BC, 2, LR]
        im2 = _shuffle_in(state_imag[rows])
        sri = np.stack([re2, im2], axis=1).astype(np.float16)  # [BC, (e a), LR]
        maps.append({"sri": sri.reshape(BC, 4, LR), "wall": wall, "gc": gc})
    return maps


def _unshuffle_out(plane2):
    # [rows, 2(c), (l r)] -> [rows, 8192] with j = (l c r)
    r = plane2.reshape(-1, 2, 32, 128).transpose(0, 2, 1, 3)
    return r.reshape(-1, N)


def kernel(state_real, state_imag, gate_real, gate_imag):
    state_real = np.asarray(state_real, dtype=np.float32)
    state_imag = np.asarray(state_imag, dtype=np.float32)
    wall, gc = _host_tensors(gate_real, gate_imag)

    nc = _get_nc()
    res = run_bass_kernel_spmd(
        nc, _in_maps(state_real, state_imag, wall, gc), list(range(NCORES))
    )

    out = np.empty((2, B, N), dtype=np.float32)
    for i in range(NCORES):
        rows = slice(i * BC, (i + 1) * BC)
        opk = res.results[i]["opk"].reshape(BC, 2, 2, LR)  # [BC, e', c, LR] f16
        out[0, rows] = _unshuffle_out(opk[:, 0].astype(np.float32))
        out[1, rows] = _unshuffle_out(opk[:, 1].astype(np.float32))
    return out
